# revision 23
# baseline (speedup 1.0000x reference)
"""Trainium2 Bass kernel for nn_Drifting_74423193305271 (cosine-similarity loss).

Reference computes, per batch b:
    x = fix_outputs * region_mask          (mask over feature dim)
    G = x @ x.T  (S x S gram), sim = G / (n n^T),  n_t = max(||x_t||, eps)
    loss terms = sum over strict upper triangle of sim, all batches
    out = -log(1 - 0.5*(avg+1)) * 0.1

Key identity: with y_t = x_t / n_t,
    sum_{t<u} sim_tu = 0.5 * (||sum_t y_t||^2 - sum_t ||y_t||^2)
so the O(S^2 D) gram matrix is never needed — one masked-norm pass over the
data plus a weighted column sum (a [1,S] @ [S,D] matmul) suffices.

Device work per core (4 batches of [512, 1024] f32):
    xm      = x * mask  (bf16)              (DVE tensor_mul; mask replicated to
                                             128 partitions via a K=1 PE matmul
                                             into PSUM, read directly from PSUM)
    n2[t]   = sum_d xm[t,d]^2               (ACT Square activation, accum_out)
    inv[t]  = 1 / max(sqrt(n2), eps)        (bf16 for the PE)
    s[d]    = sum_t inv[t] * xm[t,d]        (PE bf16 matmul, f32 PSUM accum)
    tr[t]   = n2[t] * round_bf16(inv[t])^2  (diagonal term; uses the SAME
                                             rounded inv the PE consumes so the
                                             diagonal inside ||s||^2 cancels)
Host combines: total = 0.5 * (sum mask*(s^2) - sum tr), then the log penalty.

NB: vector.tensor_tensor_reduce wedges the device (NRT INTERNAL error) on this
stack — avoid it; the tensor_mul + activation(accum_out) split above is the
working equivalent. bf16 matmul matters: fp32 PE streams at ~4 cycles/column.
"""

import sys

import numpy as np

if "/opt/trn_rl_repo" not in sys.path:
    sys.path.insert(0, "/opt/trn_rl_repo")

B, S, D = 32, 512, 1024
N_CORES = 8
B_PER = B // N_CORES  # 4 batches per core
P = 128
T_TILES = S // P  # 4 row tiles of 128 timesteps per batch
N_COLS = B_PER * T_TILES  # 16 stat columns per core
EPS = 1e-8
BETA = 0.1
H = 512  # matmul free-dim half (one PSUM bank)

_compiled_nc = None


def _build(reps: int = 1, loop_n: int = 0):
    """loop_n > 0 wraps the body in a device-side For_i loop (benchmarking
    only — one dispatch then executes the kernel loop_n * reps times)."""
    from contextlib import ExitStack, nullcontext

    import concourse.bass as bass
    import concourse.tile as tile
    from concourse import bacc, mybir

    fp32 = mybir.dt.float32
    bf16 = mybir.dt.bfloat16

    nc = bacc.Bacc(
        "TRN2",
        target_bir_lowering=False,
        debug=False,
        num_devices=N_CORES,
    )

    x_d = nc.dram_tensor("x", [B_PER * S, D], fp32, kind="ExternalInput")
    m_d = nc.dram_tensor("mask", [1, B_PER * D], fp32, kind="ExternalInput")
    s_d = nc.dram_tensor("out_s", [1, B_PER * D], fp32, kind="ExternalOutput")
    tr_d = nc.dram_tensor("out_tr", [P, N_COLS], fp32, kind="ExternalOutput")

    with tile.TileContext(nc) as tc, ExitStack() as ctx:
        x_pool = ctx.enter_context(tc.tile_pool(name="x", bufs=6))
        xm_pool = ctx.enter_context(tc.tile_pool(name="xm", bufs=3 * T_TILES))
        sq_pool = ctx.enter_context(tc.tile_pool(name="sq", bufs=3))
        const_pool = ctx.enter_context(tc.tile_pool(name="const", bufs=1))
        stat_pool = ctx.enter_context(tc.tile_pool(name="stat", bufs=2))
        mpsum_pool = ctx.enter_context(
            tc.tile_pool(name="mpsum", bufs=2, space="PSUM")
        )
        spsum_pool = ctx.enter_context(
            tc.tile_pool(name="spsum", bufs=2, space="PSUM")
        )

        ones = const_pool.tile([1, P], bf16, tag="ones")
        nc.vector.memset(ones[:], 1.0)
        mrow_f = const_pool.tile([1, B_PER * D], fp32, tag="mrow_f")
        nc.sync.dma_start(mrow_f[:], m_d[:, :])
        mrow = const_pool.tile([1, B_PER * D], bf16, tag="mrow")
        nc.vector.tensor_copy(mrow[:], mrow_f[:])  # 0/1 values: exact in bf16

        loop_cm = tc.For_i(0, loop_n, 1) if loop_n > 0 else nullcontext()
        with loop_cm:
            for _rep in range(reps):
                n2_all = stat_pool.tile([P, N_COLS], fp32, tag="n2")
                inv_all = stat_pool.tile([P, N_COLS], fp32, tag="inv")
                inv_bf = stat_pool.tile([P, N_COLS], bf16, tag="invbf")
                inv_rt = stat_pool.tile([P, N_COLS], fp32, tag="invrt")
                tr_all = stat_pool.tile([P, N_COLS], fp32, tag="tr")
                s_sb = stat_pool.tile([1, B_PER * D], fp32, tag="s_sb")
                n2c = stat_pool.tile([P, N_COLS], fp32, tag="n2c")
                nrm = stat_pool.tile([P, N_COLS], fp32, tag="nrm")
                i2 = stat_pool.tile([P, N_COLS], fp32, tag="i2")

                for b in range(B_PER):
                    # mask row -> [128, D] replica in PSUM, K=1 matmul w/ ones
                    mp = mpsum_pool.tile([P, D], fp32)
                    for h in range(2):
                        nc.tensor.matmul(
                            mp[:, h * H : (h + 1) * H],
                            ones[:, :],
                            mrow[0:1, b * D + h * H : b * D + (h + 1) * H],
                            start=True,
                            stop=True,
                        )

                    xms = []
                    for ti in range(T_TILES):
                        xt = x_pool.tile([P, D], fp32)
                        r0 = b * S + ti * P
                        nc.sync.dma_start(xt[:], x_d[r0 : r0 + P, :])

                        col = b * T_TILES + ti
                        xm = xm_pool.tile([P, D], bf16)
                        nc.vector.tensor_mul(xm[:], xt[:], mp[:])
                        xms.append(xm)
                        sq = sq_pool.tile([P, D], bf16)
                        nc.scalar.activation(
                            sq[:],
                            xm[:],
                            mybir.ActivationFunctionType.Square,
                            accum_out=n2_all[:, col : col + 1],
                        )

                    bsl = slice(b * T_TILES, (b + 1) * T_TILES)
                    nc.vector.tensor_scalar_max(
                        n2c[:, bsl], n2_all[:, bsl], EPS * EPS
                    )
                    nc.scalar.sqrt(nrm[:, bsl], n2c[:, bsl])
                    nc.vector.reciprocal(inv_all[:, bsl], nrm[:, bsl])
                    # PE consumes bf16 weights; tr must use the SAME rounded
                    # inv so the diagonal inside ||s||^2 cancels exactly.
                    nc.vector.tensor_copy(inv_bf[:, bsl], inv_all[:, bsl])
                    nc.vector.tensor_copy(inv_rt[:, bsl], inv_bf[:, bsl])
                    nc.vector.tensor_mul(
                        i2[:, bsl], inv_rt[:, bsl], inv_rt[:, bsl]
                    )
                    nc.vector.tensor_mul(
                        tr_all[:, bsl], i2[:, bsl], n2_all[:, bsl]
                    )

                    # s[d] = sum_t inv_t * xm[t,d] over the 4 row tiles
                    sp = spsum_pool.tile([1, D], fp32)
                    for ti in range(T_TILES):
                        col = b * T_TILES + ti
                        for h in range(2):
                            nc.tensor.matmul(
                                sp[0:1, h * H : (h + 1) * H],
                                inv_bf[:, col : col + 1],
                                xms[ti][:, h * H : (h + 1) * H],
                                start=(ti == 0),
                                stop=(ti == T_TILES - 1),
                            )
                    nc.scalar.copy(s_sb[0:1, b * D : (b + 1) * D], sp[0:1, :])

                nc.sync.dma_start(s_d[:, :], s_sb[:, :])
                nc.sync.dma_start(tr_d[:, :], tr_all[:, :])

    nc.compile()
    return nc


def _get_nc():
    global _compiled_nc
    if _compiled_nc is None:
        _compiled_nc = _build()
    return _compiled_nc


def _finish(mask_f32: np.ndarray, s_raws: list, trs: list) -> np.ndarray:
    """Host tail: mask s, square-sum, subtract trace, log penalty (f64)."""
    total = 0.0
    for c in range(N_CORES):
        s_raw = np.asarray(s_raws[c], dtype=np.float64).reshape(B_PER, D)
        tr = np.asarray(trs[c], dtype=np.float64)  # [P, N_COLS]
        m = mask_f32[c * B_PER : (c + 1) * B_PER].astype(np.float64)
        sm = s_raw * m
        total += 0.5 * ((sm * sm).sum() - tr.sum())
    count = B * S * (S - 1) // 2
    avg = total / count
    loss = -np.log(1.0 - 0.5 * (avg + 1.0)) * BETA
    return np.asarray(loss, dtype=np.float32)


def kernel(fix_outputs: np.ndarray, region_mask: np.ndarray) -> np.ndarray:
    from concourse.bass_utils import run_bass_kernel_spmd

    x = np.ascontiguousarray(np.asarray(fix_outputs), dtype=np.float32)
    mask_f32 = np.ascontiguousarray(np.asarray(region_mask).astype(np.float32))

    nc = _get_nc()
    in_maps = []
    for c in range(N_CORES):
        xs = x[c * B_PER : (c + 1) * B_PER].reshape(B_PER * S, D)
        ms = mask_f32[c * B_PER : (c + 1) * B_PER].reshape(1, B_PER * D)
        in_maps.append({"x": xs, "mask": ms})

    res = run_bass_kernel_spmd(nc, in_maps, list(range(N_CORES)))
    s_raws = [res.results[c]["out_s"] for c in range(N_CORES)]
    trs = [res.results[c]["out_tr"] for c in range(N_CORES)]
    return _finish(mask_f32, s_raws, trs)
